# revision 10
# baseline (speedup 1.0000x reference)
"""Trainium2 Bass kernel for nn_ConcatHandshaking.

Computes out[b, p, :] = tanh(proj_i[b, ii[p], :] + proj_j[b, jj[p], :])
where proj_i = hidden @ W[:D], proj_j = hidden @ W[D:] + bias, and (ii, jj)
are the upper-triangular token pairs of a length-S sequence.

Sharding: data-parallel over batch. B=16 batches -> 2 per core on 8 cores.

All on-device data is fp16 (host casts inputs, output cast back to f32 on
host; fp16 keeps max rel err ~2.6e-3 vs the 2e-2 gate). This halves the
dominant HBM traffic (the 8256x768 output per batch) versus f32.

Per-core pipeline:
  Stage A (both batches up front): hidden arrives pre-transposed from host
           as [D, S]; fp16 matmuls hiddenT.T x W-half -> proj_i, proj_j
           (S=128 partitions, H=768 free) accumulated in PSUM f32, bias
           folded in as a K=1 ones-vector matmul, DVE-copied to SBUF fp16.
  Stage B: pair axis (P=8256) split into 64 full tiles of 128 pairs plus a
           64-pair tail. Tile t (t<64) holds pairs 512*(t//4) + 4*m + (t%4),
           m=0..127, so each PSUM partition p of an 8-tile output group
           holds 4+4 CONSECUTIVE out rows -> 6KB contiguous DMA
           descriptors. Each tile is processed for BOTH batches under one
           selector weight load (stationary reuse: 1 LDWEIGHTS per selector
           per tile instead of per batch), accumulating
           selI.T @ proj_i + selJ.T @ proj_j into one 4-bank PSUM tile
           (batch0 at cols 0:768, batch1 at 1024:1792). One ScalarE tanh
           drains both batches' rows into an fp16 staging group; groups are
           stored with ~1.57MB DMAs (final group split for a short drain).

DMA rings: sync (SP): W1, bias, all output stores. scalar (ACT): W2 only
(ACT instruction slots are precious - ACT is a near-limiting engine).
gpsimd (SWDGE): hiddenT then fp8 selector loads. Selectors are stored fp8
in DRAM (0/1 exact) and converted to fp16 by the otherwise-idle DVE.
"""

import sys

if "/opt/trn_rl_repo" not in sys.path:
    sys.path.insert(0, "/opt/trn_rl_repo")

from contextlib import ExitStack

import numpy as np

import concourse.bass as bass
import concourse.mybir as mybir
import concourse.tile as tile
from concourse import bacc
from concourse.bass_utils import run_bass_kernel_spmd

B, S, D, H = 16, 128, 768, 768
P = S * (S + 1) // 2  # 8256 upper-triangular pairs
NCORES = 8
BPC = B // NCORES  # batches per core
NFT = 64  # full pair tiles of 128 pairs (covering 8192 pairs)
NT = NFT + 1  # plus one 64-pair tail tile
TAIL = P - NFT * S  # 64
GRP = 8  # pair tiles per output staging group (1024 pairs, ~1.57MB fp16)
NGRP = NFT // GRP  # 8 full groups per batch

F32 = mybir.dt.float32
F16 = mybir.dt.float16
FP8 = mybir.dt.float8e4

TANH = mybir.ActivationFunctionType.Tanh


def _build_nc(repeat: int = 1) -> bass.Bass:
    nc = bacc.Bacc(
        "TRN2", target_bir_lowering=False, debug=False, num_devices=NCORES
    )

    hidt = nc.declare_dram_parameter("hidt", [BPC, D, S], F16, isOutput=False)
    w = nc.declare_dram_parameter("w", [2 * D, H], F16, isOutput=False)
    bias = nc.declare_dram_parameter("bias", [1, H], F16, isOutput=False)
    sel_i = nc.declare_dram_parameter("sel_i", [S, NT, S], FP8, isOutput=False)
    sel_j = nc.declare_dram_parameter("sel_j", [S, NT, S], FP8, isOutput=False)
    out = nc.declare_dram_parameter("out", [BPC, P, H], F16, isOutput=True)

    NCH = D // 128  # 6 k-chunks

    with tile.TileContext(nc) as tc, ExitStack() as ctx:
        consts = ctx.enter_context(tc.tile_pool(name="consts", bufs=1))
        acts = ctx.enter_context(tc.tile_pool(name="acts", bufs=2))
        outs = ctx.enter_context(tc.tile_pool(name="outs", bufs=2))
        # One shared PSUM pool: every tile is [128, 2048] f32 = 4 banks;
        # bufs=2 -> all 8 banks.
        psum = ctx.enter_context(tc.tile_pool(name="psum", bufs=2, space="PSUM"))

        # ---- loads ---------------------------------------------------------
        hid_sb = []
        for bb in range(BPC):
            ht = consts.tile([128, NCH, S], F16, name=f"hidt_{bb}")
            nc.gpsimd.dma_start(ht[:], hidt[bb].rearrange("(c k) s -> k c s", k=128))
            hid_sb.append(ht)

        # Selector fp8 staging right behind hidT on the SWDGE ring; fp16
        # copies are DVE-converted group by group during the first pass.
        selib_i = consts.tile([S, NT, S], FP8, name="selib_i")
        selib_j = consts.tile([S, NT, S], FP8, name="selib_j")
        SEL_CH = 2 * GRP
        for t0c in range(0, NT, SEL_CH):
            sc = slice(t0c, min(t0c + SEL_CH, NT))
            nc.gpsimd.dma_start(selib_i[:, sc, :], sel_i[:, sc, :])
            nc.gpsimd.dma_start(selib_j[:, sc, :], sel_j[:, sc, :])
        seli_mm = consts.tile([S, NT, S], F16, name="seli_mm")
        selj_mm = consts.tile([S, NT, S], F16, name="selj_mm")

        ones_sb = consts.tile([1, 128], F16, name="ones")
        nc.gpsimd.memset(ones_sb[:], 1.0)
        warm_w = consts.tile([128, 128], F16, name="warm_w")
        nc.gpsimd.memset(warm_w[:], 0.125)

        bias_sb = consts.tile([1, H], F16, name="bias_sb")
        nc.sync.dma_start(bias_sb[:], bias[:])

        # W1 chunk 0 first and alone so stage A's first matmuls start as
        # early as possible; the rest in bigger pieces.
        w1_sb = consts.tile([128, NCH, H], F16, name="w1")
        w2_sb = consts.tile([128, NCH, H], F16, name="w2")
        for c0, cn in ((0, 1), (1, 2), (3, 3)):
            nc.sync.dma_start(
                w1_sb[:, c0 : c0 + cn, :],
                w[c0 * 128 : (c0 + cn) * 128, :].rearrange("(c k) h -> k c h", k=128),
            )
        for c0, cn in ((0, 1), (1, 2), (3, 3)):
            nc.scalar.dma_start(
                w2_sb[:, c0 : c0 + cn, :],
                w[D + c0 * 128 : D + (c0 + cn) * 128, :].rearrange(
                    "(c k) h -> k c h", k=128
                ),
            )

        # PE warmup: ~3us of junk matmuls so the HAM clock-gate opens before
        # the real matmuls arrive (cold PE runs at half rate).
        warm_ps = psum.tile([128, 2048], F32, name="warm_ps", tag="ps")
        for k in range(28):
            nc.tensor.matmul(
                warm_ps[:, 0:128], lhsT=warm_w[:], rhs=warm_w[:], start=True, stop=True
            )

        for rep in range(repeat):
            # ---- stage A: projections for both batches --------------------
            pis, pjs = [], []
            for bb in range(BPC):
                pp = psum.tile([128, 2048], F32, name=f"pp_{rep}_{bb}", tag="ps")
                for c in range(NCH):
                    first = c == 0
                    last = c == NCH - 1
                    hT = hid_sb[bb][:, c, :]
                    nc.tensor.matmul(
                        pp[:, 0:512], lhsT=hT, rhs=w1_sb[:, c, 0:512],
                        start=first, stop=last,
                    )
                    nc.tensor.matmul(
                        pp[:, 512:768], lhsT=hT, rhs=w1_sb[:, c, 512:768],
                        start=first, stop=last,
                    )
                    nc.tensor.matmul(
                        pp[:, 1024:1536], lhsT=hT, rhs=w2_sb[:, c, 0:512],
                        start=first, stop=False,
                    )
                    nc.tensor.matmul(
                        pp[:, 1536:1792], lhsT=hT, rhs=w2_sb[:, c, 512:768],
                        start=first, stop=False,
                    )
                # bias folded in as a K=1 matmul of ones.T @ b
                nc.tensor.matmul(
                    pp[:, 1024:1536], lhsT=ones_sb[:], rhs=bias_sb[:, 0:512],
                    start=False, stop=True,
                )
                nc.tensor.matmul(
                    pp[:, 1536:1792], lhsT=ones_sb[:], rhs=bias_sb[:, 512:768],
                    start=False, stop=True,
                )
                pi = acts.tile([S, H], F16, name=f"pi_{rep}_{bb}")
                pj = acts.tile([S, H], F16, name=f"pj_{rep}_{bb}")
                nc.vector.tensor_copy(pi[:], pp[:, 0:768])
                nc.vector.tensor_copy(pj[:], pp[:, 1024:1792])
                pis.append(pi)
                pjs.append(pj)

            # ---- stage B: pair tiles, both batches per selector load ------
            for g in range(NGRP):
                t0 = g * GRP
                if rep == 0:
                    sl = slice(t0, t0 + GRP)
                    nc.vector.tensor_copy(seli_mm[:, sl, :], selib_i[:, sl, :])
                    nc.vector.tensor_copy(selj_mm[:, sl, :], selib_j[:, sl, :])
                og2 = outs.tile(
                    [128, BPC, GRP, H], F16, name=f"og_{rep}_{g}", tag="og"
                )
                for tt in range(GRP):
                    t = t0 + tt
                    pq = psum.tile(
                        [128, 2048], F32, name=f"pq_{rep}_{t}", tag="ps"
                    )
                    for sel, prs, st in (
                        (seli_mm, pis, True),
                        (selj_mm, pjs, False),
                    ):
                        nc.tensor.matmul(
                            pq[:, 0:512], lhsT=sel[:, t, :],
                            rhs=prs[0][:, 0:512], start=st, stop=not st,
                        )
                        nc.tensor.matmul(
                            pq[:, 512:768], lhsT=sel[:, t, :],
                            rhs=prs[0][:, 512:768], start=st, stop=not st,
                        )
                        nc.tensor.matmul(
                            pq[:, 1024:1536], lhsT=sel[:, t, :],
                            rhs=prs[1][:, 0:512], start=st, stop=not st,
                        )
                        nc.tensor.matmul(
                            pq[:, 1536:1792], lhsT=sel[:, t, :],
                            rhs=prs[1][:, 512:768], start=st, stop=not st,
                        )
                    nc.scalar.activation(
                        og2[:, :, tt, :],
                        pq.rearrange("p (t x) -> p t x", t=2)[:, :, 0:768],
                        TANH,
                    )
                r0 = t0 * S
                for bb in range(BPC):
                    dst = out[bb, r0 : r0 + GRP * S, :].rearrange(
                        "(x p g) h -> p x g h", p=128, g=4
                    )
                    src = og2[:, bb, :, :].rearrange("p (x g) h -> p x g h", x=2)
                    if g == NGRP - 1:
                        # split the final big stores so the post-compute
                        # DMA drain is short
                        nc.sync.dma_start(dst[:, 0], src[:, 0])
                        nc.sync.dma_start(dst[:, 1], src[:, 1])
                    else:
                        nc.sync.dma_start(dst, src)
                if g == 0:
                    # ---- tail: 64 pairs via selector tile NFT, processed
                    # early so its ACT/store latency hides mid-stream ------
                    if rep == 0:
                        nc.vector.tensor_copy(
                            seli_mm[:, NFT:NT, :], selib_i[:, NFT:NT, :]
                        )
                        nc.vector.tensor_copy(
                            selj_mm[:, NFT:NT, :], selib_j[:, NFT:NT, :]
                        )
                    pqt = psum.tile([128, 2048], F32, name=f"pqt_{rep}", tag="ps")
                    for sel, prs, st in (
                        (seli_mm, pis, True),
                        (selj_mm, pjs, False),
                    ):
                        nc.tensor.matmul(
                            pqt[:, 0:512], lhsT=sel[:, NFT, :],
                            rhs=prs[0][:, 0:512], start=st, stop=not st,
                        )
                        nc.tensor.matmul(
                            pqt[:, 512:768], lhsT=sel[:, NFT, :],
                            rhs=prs[0][:, 512:768], start=st, stop=not st,
                        )
                        nc.tensor.matmul(
                            pqt[:, 1024:1536], lhsT=sel[:, NFT, :],
                            rhs=prs[1][:, 0:512], start=st, stop=not st,
                        )
                        nc.tensor.matmul(
                            pqt[:, 1536:1792], lhsT=sel[:, NFT, :],
                            rhs=prs[1][:, 512:768], start=st, stop=not st,
                        )
                    og2t = acts.tile([128, BPC, H], F16, name=f"ogt_{rep}")
                    nc.scalar.activation(
                        og2t[0:TAIL, :, :],
                        pqt.rearrange("p (t x) -> p t x", t=2)[0:TAIL, :, 0:768],
                        TANH,
                    )
                    for bb in range(BPC):
                        nc.sync.dma_start(
                            out[bb, NFT * S : P, :], og2t[0:TAIL, bb, :]
                        )

    nc.compile()
    return nc


_NC_CACHE: dict[int, bass.Bass] = {}
LAST_RESULTS = None  # BassKernelResults of the most recent kernel() call


def _get_nc(repeat: int = 1) -> bass.Bass:
    if repeat not in _NC_CACHE:
        _NC_CACHE[repeat] = _build_nc(repeat)
    return _NC_CACHE[repeat]


_SEL_CACHE = None


def _selectors() -> tuple[np.ndarray, np.ndarray]:
    """0/1 selector matrices, fp8 (exact). Tile t<64 column m selects pair
    512*(t//4) + 4*m + (t%4); tile 64 column m<64 selects pair 8192+m."""
    global _SEL_CACHE
    if _SEL_CACHE is not None:
        return _SEL_CACHE
    import ml_dtypes

    ii, jj = np.triu_indices(S)
    sel_i = np.zeros((S, NT, S), dtype=np.float32)
    sel_j = np.zeros((S, NT, S), dtype=np.float32)
    m = np.arange(S)
    for t in range(NFT):
        pr = 512 * (t // 4) + 4 * m + (t % 4)
        sel_i[ii[pr], t, m] = 1.0
        sel_j[jj[pr], t, m] = 1.0
    mt = np.arange(TAIL)
    pr = NFT * S + mt
    sel_i[ii[pr], NFT, mt] = 1.0
    sel_j[jj[pr], NFT, mt] = 1.0
    _SEL_CACHE = (
        sel_i.astype(ml_dtypes.float8_e4m3),
        sel_j.astype(ml_dtypes.float8_e4m3),
    )
    return _SEL_CACHE


def kernel(hidden: np.ndarray, W: np.ndarray, b: np.ndarray) -> np.ndarray:
    hidden = np.asarray(hidden, dtype=np.float32)
    W = np.asarray(W, dtype=np.float32)
    b = np.asarray(b, dtype=np.float32)

    sel_i, sel_j = _selectors()
    # hidden pre-transposed per batch to [D, S] so the device needs no
    # PE transpose before the projection matmuls.
    hidt = np.ascontiguousarray(hidden.transpose(0, 2, 1)).astype(np.float16)
    w16 = W.astype(np.float16)
    b16 = b.astype(np.float16).reshape(1, H)

    nc = _get_nc()
    in_maps = []
    for c in range(NCORES):
        in_maps.append(
            {
                "hidt": hidt[c * BPC : (c + 1) * BPC],
                "w": w16,
                "bias": b16,
                "sel_i": sel_i,
                "sel_j": sel_j,
            }
        )
    res = run_bass_kernel_spmd(nc, in_maps, list(range(NCORES)))
    global LAST_RESULTS
    LAST_RESULTS = res
    out = np.concatenate([res.results[c]["out"] for c in range(NCORES)], axis=0)
    return out.astype(np.float32)
